# revision 4
# baseline (speedup 1.0000x reference)
"""Gemma3 sliding-window attention (B=2, S=4096, HID=640, 4 Q heads / 1 KV head,
HD=256, window=512, softcap=50, per-head RMSNorm on Q/K, RoPE) on 8 TRN2 cores.

Sharding: sequence-parallel. 8 cores = 2 batches x 4 query-chunks of 1024
tokens. Each core computes all 4 heads for its chunk; the sliding window
means it only needs keys [qstart-512, qstart+1024) (1536 ctx rows). Output
rows are disjoint -> no collective. The [B,1,S,S] attention mask is never
shipped to the device: the host extracts the 640-wide diagonal band each
query block can see (exact for any mask supported inside the sliding window).

Per-core device pipeline (all matmuls bf16, fp32 accumulate):
  K/V proj -> K RMSNorm+RoPE (folded (1+w) tables) -> K^T via PE transpose;
  per 128-query block: Q proj -> RoPE (r_q folded into the tanh scale) ->
  Q^T -> banded scores -> tanh softcap (ACT, per-partition scale r_q/800) ->
  +band-mask -> exp (ACT, accum_out = row sums; no max subtraction needed
  since |scores| <= 50) -> P=E/den -> P^T -> head-stacked AV -> O^T -> o_proj.
"""
import sys

for _p in ("/root/.axon_site/_ro/trn_rl_repo", "/opt/trn_rl_repo"):
    if _p not in sys.path:
        sys.path.append(_p)

import numpy as np
import ml_dtypes

B, S, HID = 2, 4096, 640
NH, HD = 4, 256
W, CH, CTX = 512, 1024, 1536
NT = CH // 128           # 8 query blocks per core
NKB = CTX // 128         # 12 context blocks per core
NC_HID = HID // 128      # 5 hidden chunks
BAND = W + 128           # 640 band columns per query block
EPS = 1e-6
SOFTCAP = 50.0

_BF16 = ml_dtypes.bfloat16
_CACHE = {}


# ----------------------------------------------------------------- host prep

def _make_tables(c_, s_, w):
    """Fold (1+w) into cos/sin with the rotate-half sign convention so that
    rope(rms_scaled_x) = x*wc + shuffle(x)*ws, where shuffle swaps halves."""
    wc = c_ * (1.0 + w)[None, :]
    w_roll = np.concatenate([w[HD // 2:], w[:HD // 2]])
    sign = np.concatenate(
        [-np.ones(HD // 2, np.float32), np.ones(HD // 2, np.float32)])
    ws = s_ * (1.0 + w_roll)[None, :] * sign[None, :]
    return wc.astype(np.float32), ws.astype(np.float32)


def _prep_core(core, hidden, cos, sin, mask, q_norm_w, k_norm_w):
    b, c = core // 4, core % 4
    qs = c * CH
    lo = qs - W
    src_lo = max(lo, 0)

    hctx = np.zeros((CTX, HID), np.float32)
    hctx[src_lo - lo:] = hidden[b, src_lo: qs + CH]

    ck = np.zeros((CTX, HD), np.float32)
    sk = np.zeros((CTX, HD), np.float32)
    ck[src_lo - lo:] = cos[0, src_lo: qs + CH]
    sk[src_lo - lo:] = sin[0, src_lo: qs + CH]

    wcq, wsq = _make_tables(cos[0, qs:qs + CH], sin[0, qs:qs + CH], q_norm_w)
    wck, wsk = _make_tables(ck, sk, k_norm_w)

    bm = np.full((CH, BAND), -2e7, np.float32)
    for t in range(NT):
        q0 = qs + t * 128
        j_lo = q0 - W
        jsrc_lo = max(j_lo, 0)
        bm[t * 128:(t + 1) * 128, jsrc_lo - j_lo:] = (
            mask[b, 0, q0:q0 + 128, jsrc_lo:q0 + 128] / SOFTCAP)

    return {
        "ht": np.ascontiguousarray(hctx.T).astype(_BF16),
        "wcq": wcq, "wsq": wsq, "wck": wck, "wsk": wsk, "bmask": bm,
    }


# -------------------------------------------------------------- device build

def _build_module():
    if "nc" in _CACHE:
        return _CACHE["nc"]

    from contextlib import ExitStack
    import concourse.bass as bass
    import concourse.mybir as mybir
    from concourse import bacc
    from concourse.tile import TileContext
    from concourse.masks import make_identity

    f32 = mybir.dt.float32
    bf16 = mybir.dt.bfloat16
    AF = mybir.ActivationFunctionType
    Alu = mybir.AluOpType

    nc = bacc.Bacc("TRN2", target_bir_lowering=False)

    ht_d = nc.dram_tensor("ht", [HID, CTX], bf16, kind="ExternalInput")
    wq_d = nc.dram_tensor("wq", [HID, NH * HD], bf16, kind="ExternalInput")
    wk_d = nc.dram_tensor("wk", [HID, HD], bf16, kind="ExternalInput")
    wv_d = nc.dram_tensor("wv", [HID, HD], bf16, kind="ExternalInput")
    wo_d = nc.dram_tensor("wo", [NH * HD, HID], bf16, kind="ExternalInput")
    wcq_d = nc.dram_tensor("wcq", [CH, HD], f32, kind="ExternalInput")
    wsq_d = nc.dram_tensor("wsq", [CH, HD], f32, kind="ExternalInput")
    wck_d = nc.dram_tensor("wck", [CTX, HD], f32, kind="ExternalInput")
    wsk_d = nc.dram_tensor("wsk", [CTX, HD], f32, kind="ExternalInput")
    bm_d = nc.dram_tensor("bmask", [CH, BAND], f32, kind="ExternalInput")
    out_d = nc.dram_tensor("out", [CH, HID], f32, kind="ExternalOutput")

    with TileContext(nc) as tc, ExitStack() as ctx:
        singles = ctx.enter_context(tc.tile_pool(name="singles", bufs=1))
        pool_w = ctx.enter_context(tc.tile_pool(name="work", bufs=3))
        pool_st = ctx.enter_context(tc.tile_pool(name="stats", bufs=8))
        pool_pt = ctx.enter_context(tc.tile_pool(name="ptall", bufs=2))
        pool_ot = ctx.enter_context(tc.tile_pool(name="otsb", bufs=2))
        pool_out = ctx.enter_context(tc.tile_pool(name="outsb", bufs=2))
        pool_mm = ctx.enter_context(
            tc.tile_pool(name="psmm", bufs=5, space="PSUM"))
        pool_s = ctx.enter_context(
            tc.tile_pool(name="psscore", bufs=1, space="PSUM"))
        pool_t = ctx.enter_context(
            tc.tile_pool(name="pstrans", bufs=1, space="PSUM"))

        # resident tensors
        ht_sb = singles.tile([128, NC_HID, CTX], bf16)
        wq_sb = singles.tile([128, NC_HID, NH * HD], bf16)
        wk_sb = singles.tile([128, NC_HID, HD], bf16)
        wv_sb = singles.tile([128, NC_HID, HD], bf16)
        wo_sb = singles.tile([128, 2 * NH, HID], bf16)
        wcq_sb = singles.tile([128, NT, HD], f32)
        wsq_sb = singles.tile([128, NT, HD], f32)
        wck_sb = singles.tile([128, NKB, HD], f32)
        wsk_sb = singles.tile([128, NKB, HD], f32)
        bm_sb = singles.tile([128, NT, BAND], f32)
        kt_sb = singles.tile([128, 2, CTX], bf16)
        v_sb = singles.tile([128, NKB, HD], bf16)
        ident = singles.tile([128, 128], bf16)
        eps_k = singles.tile([128, 1], f32)
        eps_q = singles.tile([128, 1], f32)

        make_identity(nc, ident)
        nc.vector.memset(eps_k, EPS)
        nc.vector.memset(eps_q, 640000.0 * EPS)

        for c in range(NC_HID):
            nc.sync.dma_start(out=ht_sb[:, c, :], in_=ht_d[c * 128:(c + 1) * 128, :])
            nc.sync.dma_start(out=wq_sb[:, c, :], in_=wq_d[c * 128:(c + 1) * 128, :])
            nc.sync.dma_start(out=wk_sb[:, c, :], in_=wk_d[c * 128:(c + 1) * 128, :])
            nc.sync.dma_start(out=wv_sb[:, c, :], in_=wv_d[c * 128:(c + 1) * 128, :])
        for j in range(2 * NH):
            nc.sync.dma_start(out=wo_sb[:, j, :], in_=wo_d[j * 128:(j + 1) * 128, :])
        for t in range(NT):
            nc.sync.dma_start(out=wcq_sb[:, t, :], in_=wcq_d[t * 128:(t + 1) * 128, :])
            nc.sync.dma_start(out=wsq_sb[:, t, :], in_=wsq_d[t * 128:(t + 1) * 128, :])
            nc.sync.dma_start(out=bm_sb[:, t, :], in_=bm_d[t * 128:(t + 1) * 128, :])
        for k in range(NKB):
            nc.sync.dma_start(out=wck_sb[:, k, :], in_=wck_d[k * 128:(k + 1) * 128, :])
            nc.sync.dma_start(out=wsk_sb[:, k, :], in_=wsk_d[k * 128:(k + 1) * 128, :])

        # ---------------- phase 1: K / V over the 1536-row context ----------
        for kb in range(NKB):
            kp = pool_mm.tile([128, HD], f32, tag="mm", name="kp")
            for c in range(NC_HID):
                nc.tensor.matmul(
                    kp, ht_sb[:, c, kb * 128:(kb + 1) * 128], wk_sb[:, c, :],
                    start=(c == 0), stop=(c == NC_HID - 1))

            ssk = pool_st.tile([128, 1], f32, tag="ssk", name="ssk")
            sqk = pool_w.tile([128, HD], f32, tag="sq", name="sqk")
            nc.scalar.activation(out=sqk, in_=kp, func=AF.Square, accum_out=ssk)
            sk1 = pool_st.tile([128, 1], f32, tag="sk1", name="sk1")
            nc.scalar.activation(out=sk1, in_=ssk, func=AF.Sqrt,
                                 scale=1.0 / HD, bias=eps_k)
            rk = pool_st.tile([128, 1], f32, tag="rk", name="rk")
            nc.vector.reciprocal(rk, sk1)

            # rope with r_k folded in: u=(kp*rk)*wck; v=(swap(kp)*rk)*wsk; y=u+v
            u = pool_w.tile([128, HD], f32, tag="u", name="uk")
            nc.vector.scalar_tensor_tensor(
                out=u, in0=kp, scalar=rk, in1=wck_sb[:, kb, :],
                op0=Alu.mult, op1=Alu.mult)
            v = pool_w.tile([128, HD], f32, tag="v", name="vk")
            nc.vector.scalar_tensor_tensor(
                out=v[:, :HD // 2], in0=kp[:, HD // 2:], scalar=rk,
                in1=wsk_sb[:, kb, :HD // 2], op0=Alu.mult, op1=Alu.mult)
            nc.vector.scalar_tensor_tensor(
                out=v[:, HD // 2:], in0=kp[:, :HD // 2], scalar=rk,
                in1=wsk_sb[:, kb, HD // 2:], op0=Alu.mult, op1=Alu.mult)
            krm = pool_w.tile([128, HD], bf16, tag="krm", name="krm")
            nc.vector.tensor_add(krm, u, v)

            for dc in range(2):
                tp = pool_t.tile([128, 128], bf16, tag="tp", name="tpk")
                nc.tensor.transpose(tp, krm[:, dc * 128:(dc + 1) * 128], ident)
                nc.vector.tensor_copy(
                    kt_sb[:, dc, kb * 128:(kb + 1) * 128], tp)

            vp = pool_mm.tile([128, HD], f32, tag="mm", name="vp")
            for c in range(NC_HID):
                nc.tensor.matmul(
                    vp, ht_sb[:, c, kb * 128:(kb + 1) * 128], wv_sb[:, c, :],
                    start=(c == 0), stop=(c == NC_HID - 1))
            nc.scalar.copy(v_sb[:, kb, :], vp)

        # ---------------- phase 2: per query block ---------------------------
        for t in range(NT):
            qcol = W + t * 128  # query rows inside the ctx (ht columns)
            qp = []
            for hp in range(2):
                q = pool_mm.tile([128, 512], f32, tag="mm", name="qp")
                for c in range(NC_HID):
                    nc.tensor.matmul(
                        q, ht_sb[:, c, qcol:qcol + 128],
                        wq_sb[:, c, hp * 512:(hp + 1) * 512],
                        start=(c == 0), stop=(c == NC_HID - 1))
                qp.append(q)

            pt_all = pool_pt.tile([128, 5, NH, 128], bf16, name="pt_all")

            for h in range(NH):
                seg = qp[h // 2][:, (h % 2) * HD:(h % 2) * HD + HD]

                ssq = pool_st.tile([128, 1], f32, tag="ssq", name="ssq")
                sqq = pool_w.tile([128, HD], f32, tag="sq", name="sqq")
                nc.scalar.activation(out=sqq, in_=seg, func=AF.Square,
                                     accum_out=ssq)
                # tanh scale = r_q/800 -> 1/sqrt(800^2/HD * ssq + 800^2*eps)
                s1 = pool_st.tile([128, 1], f32, tag="s1", name="s1")
                nc.scalar.activation(out=s1, in_=ssq, func=AF.Sqrt,
                                     scale=640000.0 / HD, bias=eps_q)
                rq = pool_st.tile([128, 1], f32, tag="rq", name="rq")
                nc.vector.reciprocal(rq, s1)

                u = pool_w.tile([128, HD], f32, tag="u", name="uq")
                nc.vector.tensor_mul(u, seg, wcq_sb[:, t, :])
                v = pool_w.tile([128, HD], f32, tag="v", name="vq")
                nc.vector.tensor_mul(
                    v[:, :HD // 2], seg[:, HD // 2:], wsq_sb[:, t, :HD // 2])
                nc.vector.tensor_mul(
                    v[:, HD // 2:], seg[:, :HD // 2], wsq_sb[:, t, HD // 2:])
                qro = pool_w.tile([128, HD], bf16, tag="qro", name="qro")
                nc.vector.tensor_add(qro, u, v)

                qt = pool_w.tile([128, 2, 128], bf16, tag="qt", name="qt")
                for dc in range(2):
                    tp = pool_t.tile([128, 128], bf16, tag="tp", name="tpq")
                    nc.tensor.transpose(tp, qro[:, dc * 128:(dc + 1) * 128], ident)
                    nc.vector.tensor_copy(qt[:, dc, :], tp)

                sp = pool_s.tile([128, BAND], f32, name="sp")
                for dc in range(2):
                    nc.tensor.matmul(
                        sp[:, :512], qt[:, dc, :],
                        kt_sb[:, dc, t * 128:t * 128 + 512],
                        start=(dc == 0), stop=(dc == 1))
                    nc.tensor.matmul(
                        sp[:, 512:], qt[:, dc, :],
                        kt_sb[:, dc, t * 128 + 512:t * 128 + BAND],
                        start=(dc == 0), stop=(dc == 1))

                tt = pool_w.tile([128, BAND], f32, tag="T", name="tt")
                nc.scalar.activation(out=tt, in_=sp, func=AF.Tanh, scale=rq)
                nc.vector.tensor_add(tt, tt, bm_sb[:, t, :])
                ee = pool_w.tile([128, BAND], f32, tag="E", name="ee")
                den = pool_st.tile([128, 1], f32, tag="den", name="den")
                nc.scalar.activation(out=ee, in_=tt, func=AF.Exp,
                                     scale=SOFTCAP, accum_out=den)
                rd = pool_st.tile([128, 1], f32, tag="rd", name="rd")
                nc.vector.reciprocal(rd, den)
                pp = pool_w.tile([128, BAND], bf16, tag="P", name="pp")
                nc.vector.tensor_scalar_mul(pp, in0=ee, scalar1=rd)

                for ci in range(5):
                    tp = pool_t.tile([128, 128], bf16, tag="tp", name="tpp")
                    nc.tensor.transpose(tp, pp[:, ci * 128:(ci + 1) * 128], ident)
                    nc.scalar.copy(pt_all[:, ci, h, :], tp)

            # head-stacked AV: O^T chunks [d-half, (h,q)]
            ot = pool_ot.tile([128, 2 * NH, 128], bf16, name="ot")
            for dc in range(2):
                avp = pool_mm.tile([128, 512], f32, tag="mm", name="avp")
                for ci in range(5):
                    nc.tensor.matmul(
                        avp, v_sb[:, t + ci, dc * 128:(dc + 1) * 128],
                        pt_all[:, ci, :, :],
                        start=(ci == 0), stop=(ci == 4))
                for h in range(NH):
                    nc.vector.tensor_copy(
                        ot[:, h * 2 + dc, :], avp[:, h * 128:(h + 1) * 128])

            outsb = pool_out.tile([128, HID], f32, name="outsb")
            for half, (n0, nsz) in enumerate(((0, 512), (512, 128))):
                op = pool_mm.tile([128, nsz], f32, tag="mm", name="op")
                for j in range(2 * NH):
                    nc.tensor.matmul(
                        op, ot[:, j, :], wo_sb[:, j, n0:n0 + nsz],
                        start=(j == 0), stop=(j == 2 * NH - 1))
                nc.scalar.copy(outsb[:, n0:n0 + nsz], op)
            nc.sync.dma_start(out=out_d[t * 128:(t + 1) * 128, :], in_=outsb)

    nc.compile()
    _CACHE["nc"] = nc
    return nc


# ------------------------------------------------------------------- kernel

def kernel(hidden_states, cos, sin, attention_mask, Wq, Wk, Wv, Wo,
           q_norm_w, k_norm_w):
    from concourse.bass_utils import run_bass_kernel_spmd

    hidden_states = np.asarray(hidden_states, np.float32)
    cos = np.asarray(cos, np.float32)
    sin = np.asarray(sin, np.float32)
    attention_mask = np.asarray(attention_mask, np.float32)
    q_norm_w = np.asarray(q_norm_w, np.float32)
    k_norm_w = np.asarray(k_norm_w, np.float32)

    wq_b = np.asarray(Wq, np.float32).astype(_BF16)
    wk_b = np.asarray(Wk, np.float32).astype(_BF16)
    wv_b = np.asarray(Wv, np.float32).astype(_BF16)
    wo_b = np.asarray(Wo, np.float32).astype(_BF16)

    in_maps = []
    for core in range(8):
        p = _prep_core(core, hidden_states, cos, sin, attention_mask,
                       q_norm_w, k_norm_w)
        in_maps.append({
            "ht": p["ht"], "wq": wq_b, "wk": wk_b, "wv": wv_b, "wo": wo_b,
            "wcq": p["wcq"], "wsq": p["wsq"],
            "wck": p["wck"], "wsk": p["wsk"], "bmask": p["bmask"],
        })

    nc = _build_module()
    res = run_bass_kernel_spmd(nc, in_maps, core_ids=list(range(8)))

    out = np.empty((B, S, HID), np.float32)
    for core in range(8):
        b, c = core // 4, core % 4
        out[b, c * CH:(c + 1) * CH] = res.results[core]["out"]
    return out


# revision 5
# speedup vs baseline: 2.1663x; 2.1663x over previous
"""Gemma3 sliding-window attention (B=2, S=4096, HID=640, 4 Q heads / 1 KV head,
HD=256, window=512, softcap=50, per-head RMSNorm on Q/K, RoPE) on 8 TRN2 cores.

Sharding: sequence-parallel. 8 cores = 2 batches x 4 query-chunks of 1024
tokens. Each core computes all 4 heads for its chunk; the sliding window
means it only needs keys [qstart-512, qstart+1024) (1536 ctx rows). Output
rows are disjoint -> no collective. The [B,1,S,S] attention mask is never
shipped to the device: the host extracts the 640-wide diagonal band each
query block can see (exact for any mask supported inside the sliding window).

Device pipeline per core (matmuls bf16, fp32 accumulate):
  A: K/V proj over ctx; roped-unnormalized K cached; all K rms sums batched
     into ONE Sqrt (the ACT sqrt table lives in a different act-func set than
     tanh/exp — batching avoids ~1.3us table reloads per use);
  B: Q proj + RoPE + Q^T for all blocks; all 32 rms sums -> ONE Sqrt;
     r_q folds into the tanh scale (rope/scores are linear in q).
  C: per (block, head): banded scores -> tanh softcap (per-partition scale
     r_q/800) -> +band-mask -> exp (accum_out = row sums; |scores|<=50 so no
     max subtraction) -> P=E/den -> P^T -> head-stacked AV -> O^T -> o_proj.
"""
import sys

for _p in ("/root/.axon_site/_ro/trn_rl_repo", "/opt/trn_rl_repo"):
    if _p not in sys.path:
        sys.path.append(_p)

import numpy as np
import ml_dtypes

B, S, HID = 2, 4096, 640
NH, HD = 4, 256
W, CH, CTX = 512, 1024, 1536
NT = CH // 128           # 8 query blocks per core
NKB = CTX // 128         # 12 context blocks per core
NCH = HID // 128         # 5 hidden chunks
BAND = W + 128           # 640 band columns per query block
EPS = 1e-6
SOFTCAP = 50.0

# packed bf16 input layout: per-partition element offsets
_OFF_HT = 0
_OFF_WQ = _OFF_HT + NCH * CTX          # 7680
_OFF_WK = _OFF_WQ + NCH * NH * HD      # 12800
_OFF_WV = _OFF_WK + NCH * HD           # 14080
_OFF_WO = _OFF_WV + NCH * HD           # 15360
_OFF_WCQ = _OFF_WO + 2 * NH * HID      # 20480
_OFF_WSQ = _OFF_WCQ + NT * HD          # 22528
_OFF_WCK = _OFF_WSQ + NT * HD          # 24576
_OFF_WSK = _OFF_WCK + NKB * HD         # 27648
_PB_LEN = _OFF_WSK + NKB * HD          # 30720
_PF_LEN = NT * BAND                    # 5120

_BF16 = ml_dtypes.bfloat16
_CACHE = {}


# ----------------------------------------------------------------- host prep

def _pm(a, chunks):
    """[chunks*128, F] -> partition-major [128, chunks*F]."""
    a = np.ascontiguousarray(a)
    return a.reshape(chunks, 128, -1).transpose(1, 0, 2).reshape(128, -1)


def _make_tables(c_, s_, w):
    """Fold (1+w) into cos/sin with the rotate-half sign convention so that
    rope(rms_scaled_x) = x*wc + shuffle(x)*ws, where shuffle swaps halves."""
    wc = c_ * (1.0 + w)[None, :]
    w_roll = np.concatenate([w[HD // 2:], w[:HD // 2]])
    sign = np.concatenate(
        [-np.ones(HD // 2, np.float32), np.ones(HD // 2, np.float32)])
    ws = s_ * (1.0 + w_roll)[None, :] * sign[None, :]
    return wc.astype(np.float32), ws.astype(np.float32)


def _prep_core(core, hidden, cos, sin, mask, q_norm_w, k_norm_w, wtail):
    b, c = core // 4, core % 4
    qs = c * CH
    lo = qs - W
    src_lo = max(lo, 0)

    hctx = np.zeros((CTX, HID), np.float32)
    hctx[src_lo - lo:] = hidden[b, src_lo: qs + CH]

    ck = np.zeros((CTX, HD), np.float32)
    sk = np.zeros((CTX, HD), np.float32)
    ck[src_lo - lo:] = cos[0, src_lo: qs + CH]
    sk[src_lo - lo:] = sin[0, src_lo: qs + CH]

    wcq, wsq = _make_tables(cos[0, qs:qs + CH], sin[0, qs:qs + CH], q_norm_w)
    wck, wsk = _make_tables(ck, sk, k_norm_w)

    bm = np.full((CH, BAND), -2e7, np.float32)
    for t in range(NT):
        q0 = qs + t * 128
        j_lo = q0 - W
        jsrc_lo = max(j_lo, 0)
        bm[t * 128:(t + 1) * 128, jsrc_lo - j_lo:] = (
            mask[b, 0, q0:q0 + 128, jsrc_lo:q0 + 128] / SOFTCAP)

    pb = np.concatenate(
        [_pm(hctx.T, NCH).astype(_BF16), wtail,
         _pm(wcq, NT).astype(_BF16), _pm(wsq, NT).astype(_BF16),
         _pm(wck, NKB).astype(_BF16), _pm(wsk, NKB).astype(_BF16)], axis=1)
    pf = _pm(bm, NT).astype(np.float32)
    return pb, pf


def _build_inmaps(hidden_states, cos, sin, attention_mask, Wq, Wk, Wv, Wo,
                  q_norm_w, k_norm_w):
    hidden_states = np.asarray(hidden_states, np.float32)
    cos = np.asarray(cos, np.float32)
    sin = np.asarray(sin, np.float32)
    attention_mask = np.asarray(attention_mask, np.float32)
    q_norm_w = np.asarray(q_norm_w, np.float32)
    k_norm_w = np.asarray(k_norm_w, np.float32)

    wtail = np.concatenate(
        [_pm(np.asarray(Wq, np.float32), NCH),
         _pm(np.asarray(Wk, np.float32), NCH),
         _pm(np.asarray(Wv, np.float32), NCH),
         _pm(np.asarray(Wo, np.float32), 2 * NH)], axis=1).astype(_BF16)

    in_maps = []
    for core in range(8):
        pb, pf = _prep_core(core, hidden_states, cos, sin, attention_mask,
                            q_norm_w, k_norm_w, wtail)
        assert pb.shape == (128, _PB_LEN) and pf.shape == (128, _PF_LEN)
        in_maps.append({"pb": pb, "pf": pf})
    return in_maps


# -------------------------------------------------------------- device build

def _build_module():
    if "nc" in _CACHE:
        return _CACHE["nc"]

    from contextlib import ExitStack
    import concourse.mybir as mybir
    from concourse import bacc
    from concourse.tile import TileContext
    from concourse.masks import make_identity

    f32 = mybir.dt.float32
    bf16 = mybir.dt.bfloat16
    AF = mybir.ActivationFunctionType

    nc = bacc.Bacc("TRN2", target_bir_lowering=False)

    pb_d = nc.dram_tensor("pb", [128, _PB_LEN], bf16, kind="ExternalInput")
    pf_d = nc.dram_tensor("pf", [128, _PF_LEN], f32, kind="ExternalInput")
    out_d = nc.dram_tensor("out", [CH, HID], f32, kind="ExternalOutput")

    with TileContext(nc) as tc, ExitStack() as ctx:
        singles = ctx.enter_context(tc.tile_pool(name="singles", bufs=1))
        pool_w = ctx.enter_context(tc.tile_pool(name="work", bufs=3))
        pool_st = ctx.enter_context(tc.tile_pool(name="stats", bufs=8))
        pool_pt = ctx.enter_context(tc.tile_pool(name="ptall", bufs=2))
        pool_ot = ctx.enter_context(tc.tile_pool(name="otsb", bufs=2))
        pool_out = ctx.enter_context(tc.tile_pool(name="outsb", bufs=2))
        pool_mm = ctx.enter_context(
            tc.tile_pool(name="psmm", bufs=2, space="PSUM"))
        pool_s = ctx.enter_context(
            tc.tile_pool(name="psscore", bufs=2, space="PSUM"))
        pool_t = ctx.enter_context(
            tc.tile_pool(name="pstrans", bufs=2, space="PSUM"))

        pb_sb = singles.tile([128, _PB_LEN], bf16)
        pf_sb = singles.tile([128, _PF_LEN], f32)
        kt_sb = singles.tile([128, 2, CTX], bf16)
        v_sb = singles.tile([128, NKB, HD], bf16)
        ku_all = singles.tile([128, NKB, HD], f32)
        qt_all = singles.tile([128, NT, NH, 2, 128], bf16)
        ssk_all = singles.tile([128, NKB], f32)
        ssq_all = singles.tile([128, NT * NH], f32)
        rk_all = singles.tile([128, NKB], f32)
        rq_all = singles.tile([128, NT * NH], f32)
        ident = singles.tile([128, 128], bf16)
        eps_k = singles.tile([128, 1], f32)
        eps_q = singles.tile([128, 1], f32)

        make_identity(nc, ident)
        nc.vector.memset(eps_k, EPS)
        nc.vector.memset(eps_q, 640000.0 * EPS)

        # packed loads: 4 big DMAs for bf16, 1 for the f32 band mask
        qtr = _PB_LEN // 4
        for i in range(4):
            nc.sync.dma_start(out=pb_sb[:, i * qtr:(i + 1) * qtr],
                              in_=pb_d[:, i * qtr:(i + 1) * qtr])
        nc.sync.dma_start(out=pf_sb, in_=pf_d[:, :])

        def view(off, n, a):
            return pb_sb[:, off:off + n].rearrange("p (a b) -> p a b", a=a)

        ht_v = view(_OFF_HT, NCH * CTX, NCH)
        wq_v = view(_OFF_WQ, NCH * NH * HD, NCH)
        wk_v = view(_OFF_WK, NCH * HD, NCH)
        wv_v = view(_OFF_WV, NCH * HD, NCH)
        wo_v = view(_OFF_WO, 2 * NH * HID, 2 * NH)
        wcq_v = view(_OFF_WCQ, NT * HD, NT)
        wsq_v = view(_OFF_WSQ, NT * HD, NT)
        wck_v = view(_OFF_WCK, NKB * HD, NKB)
        wsk_v = view(_OFF_WSK, NKB * HD, NKB)
        bm_v = pf_sb.rearrange("p (t c) -> p t c", t=NT)

        H2 = HD // 2

        # ---------------- phase A: K / V over the 1536-row context ----------
        for kb in range(NKB):
            kp = pool_mm.tile([128, HD], f32, tag="mm", name="kp")
            for c in range(NCH):
                nc.tensor.matmul(
                    kp, ht_v[:, c, kb * 128:(kb + 1) * 128], wk_v[:, c, :],
                    start=(c == 0), stop=(c == NCH - 1))

            sqs = pool_w.tile([128, HD], f32, tag="sq", name="sqs")
            nc.scalar.activation(out=sqs, in_=kp, func=AF.Square,
                                 accum_out=ssk_all[:, kb:kb + 1])

            # unnormalized rope: u = kp*wck ; v = swap(kp)*wsk ; ku = u+v
            u = pool_w.tile([128, HD], f32, tag="u", name="uk")
            nc.vector.tensor_mul(u, kp, wck_v[:, kb, :])
            v = pool_w.tile([128, HD], f32, tag="v", name="vk")
            nc.vector.tensor_mul(v[:, :H2], kp[:, H2:], wsk_v[:, kb, :H2])
            nc.vector.tensor_mul(v[:, H2:], kp[:, :H2], wsk_v[:, kb, H2:])
            nc.vector.tensor_add(ku_all[:, kb, :], u, v)

            vp = pool_mm.tile([128, HD], f32, tag="mm", name="vp")
            for c in range(NCH):
                nc.tensor.matmul(
                    vp, ht_v[:, c, kb * 128:(kb + 1) * 128], wv_v[:, c, :],
                    start=(c == 0), stop=(c == NCH - 1))
            nc.scalar.copy(v_sb[:, kb, :], vp)

        sk_all = pool_st.tile([128, NKB], f32, tag="skal", name="sk_all")
        nc.scalar.activation(out=sk_all, in_=ssk_all, func=AF.Sqrt,
                             scale=1.0 / HD, bias=eps_k)
        nc.vector.reciprocal(rk_all, sk_all)

        for kb in range(NKB):
            krm = pool_w.tile([128, HD], bf16, tag="krm", name="krm")
            nc.vector.tensor_scalar_mul(
                krm, in0=ku_all[:, kb, :], scalar1=rk_all[:, kb:kb + 1])
            for dc in range(2):
                tp = pool_t.tile([128, 128], bf16, tag="tp", name="tpk")
                nc.tensor.transpose(tp, krm[:, dc * 128:(dc + 1) * 128], ident)
                nc.vector.tensor_copy(
                    kt_sb[:, dc, kb * 128:(kb + 1) * 128], tp)

        # ---------------- phase B: Q proj + rope + Q^T -----------------------
        for t in range(NT):
            qcol = W + t * 128
            for hp in range(2):
                qp = pool_mm.tile([128, 512], f32, tag="mm", name="qp")
                for c in range(NCH):
                    nc.tensor.matmul(
                        qp, ht_v[:, c, qcol:qcol + 128],
                        wq_v[:, c, hp * 512:(hp + 1) * 512],
                        start=(c == 0), stop=(c == NCH - 1))
                for hh in range(2):
                    h = hp * 2 + hh
                    idx = t * NH + h
                    seg = qp[:, hh * HD:(hh + 1) * HD]
                    sqs = pool_w.tile([128, HD], f32, tag="sq", name="sqq")
                    nc.scalar.activation(out=sqs, in_=seg, func=AF.Square,
                                         accum_out=ssq_all[:, idx:idx + 1])
                    u = pool_w.tile([128, HD], f32, tag="u", name="uq")
                    nc.vector.tensor_mul(u, seg, wcq_v[:, t, :])
                    v = pool_w.tile([128, HD], f32, tag="v", name="vq")
                    nc.vector.tensor_mul(v[:, :H2], seg[:, H2:], wsq_v[:, t, :H2])
                    nc.vector.tensor_mul(v[:, H2:], seg[:, :H2], wsq_v[:, t, H2:])
                    qro = pool_w.tile([128, HD], bf16, tag="qro", name="qro")
                    nc.vector.tensor_add(qro, u, v)
                    for dc in range(2):
                        tp = pool_t.tile([128, 128], bf16, tag="tp", name="tpq")
                        nc.tensor.transpose(
                            tp, qro[:, dc * 128:(dc + 1) * 128], ident)
                        nc.vector.tensor_copy(qt_all[:, t, h, dc, :], tp)

        sq1 = pool_st.tile([128, NT * NH], f32, tag="sq1", name="sq1")
        nc.scalar.activation(out=sq1, in_=ssq_all, func=AF.Sqrt,
                             scale=640000.0 / HD, bias=eps_q)
        nc.vector.reciprocal(rq_all, sq1)

        # ---------------- phase C: attention per query block -----------------
        for t in range(NT):
            pt_all = pool_pt.tile([128, 5, NH, 128], bf16, name="pt_all")
            pps = []
            for h in range(NH):
                idx = t * NH + h
                sp = pool_s.tile([128, BAND], f32, tag="sp", name="sp")
                for dc in range(2):
                    nc.tensor.matmul(
                        sp[:, :512], qt_all[:, t, h, dc, :],
                        kt_sb[:, dc, t * 128:t * 128 + 512],
                        start=(dc == 0), stop=(dc == 1))
                    nc.tensor.matmul(
                        sp[:, 512:], qt_all[:, t, h, dc, :],
                        kt_sb[:, dc, t * 128 + 512:t * 128 + BAND],
                        start=(dc == 0), stop=(dc == 1))

                tt = pool_w.tile([128, BAND], f32, tag="T", name="tt")
                nc.scalar.activation(out=tt, in_=sp, func=AF.Tanh,
                                     scale=rq_all[:, idx:idx + 1])
                nc.vector.tensor_add(tt, tt, bm_v[:, t, :])
                ee = pool_w.tile([128, BAND], f32, tag="E", name="ee")
                den = pool_st.tile([128, 1], f32, tag="den", name="den")
                nc.scalar.activation(out=ee, in_=tt, func=AF.Exp,
                                     scale=SOFTCAP, accum_out=den)
                rd = pool_st.tile([128, 1], f32, tag="rd", name="rd")
                nc.vector.reciprocal(rd, den)
                pp = pool_w.tile([128, BAND], bf16, tag="P", bufs=5, name="pp")
                nc.vector.tensor_scalar_mul(pp, in0=ee, scalar1=rd)
                pps.append(pp)

            for h in range(NH):
                for ci in range(5):
                    tp = pool_t.tile([128, 128], bf16, tag="tp", name="tpp")
                    nc.tensor.transpose(
                        tp, pps[h][:, ci * 128:(ci + 1) * 128], ident)
                    nc.vector.tensor_copy(pt_all[:, ci, h, :], tp)

            ot = pool_ot.tile([128, 2 * NH, 128], bf16, name="ot")
            for dc in range(2):
                avp = pool_mm.tile([128, 512], f32, tag="mm", name="avp")
                for ci in range(5):
                    nc.tensor.matmul(
                        avp, v_sb[:, t + ci, dc * 128:(dc + 1) * 128],
                        pt_all[:, ci, :, :],
                        start=(ci == 0), stop=(ci == 4))
                for h in range(NH):
                    nc.scalar.copy(ot[:, h * 2 + dc, :],
                                   avp[:, h * 128:(h + 1) * 128])

            outsb = pool_out.tile([128, HID], f32, name="outsb")
            for n0, nsz in ((0, 512), (512, 128)):
                op = pool_mm.tile([128, nsz], f32, tag="mm", name="op")
                for j in range(2 * NH):
                    nc.tensor.matmul(
                        op, ot[:, j, :], wo_v[:, j, n0:n0 + nsz],
                        start=(j == 0), stop=(j == 2 * NH - 1))
                nc.scalar.copy(outsb[:, n0:n0 + nsz], op)
            nc.sync.dma_start(out=out_d[t * 128:(t + 1) * 128, :], in_=outsb)

    nc.compile()
    _CACHE["nc"] = nc
    return nc


# ------------------------------------------------------------------- kernel

def kernel(hidden_states, cos, sin, attention_mask, Wq, Wk, Wv, Wo,
           q_norm_w, k_norm_w):
    from concourse.bass_utils import run_bass_kernel_spmd

    in_maps = _build_inmaps(hidden_states, cos, sin, attention_mask,
                            Wq, Wk, Wv, Wo, q_norm_w, k_norm_w)
    nc = _build_module()
    res = run_bass_kernel_spmd(nc, in_maps, core_ids=list(range(8)))

    out = np.empty((B, S, HID), np.float32)
    for core in range(8):
        b, c = core // 4, core % 4
        out[b, c * CH:(c + 1) * CH] = res.results[core]["out"]
    return out
